# revision 26
# baseline (speedup 1.0000x reference)
"""Bass/Trainium2 kernel for NF4-dequant (QLoRA-style) SwiGLU MLP.

Computation (matches the bitsandbytes-NF4 reference):
    dq_i = nf4_quant_dequant(w_i)   (per-64-block absmax scaling)
    out  = dq3-proj( silu(x @ dq1^T) * (x @ dq2^T) )

Sharding: tensor-parallel over the ffn dim H=11008 across 8 cores.
H is split in 64-aligned shards of width [1408 x4, 1344 x4]; the 1344
shards are zero-padded to 1408 so every core runs the same program.
Each core computes a full [T, D] partial of the down-projection; the
host sums the 8 partials (the TP all-reduce).

Dequant strategy (per 128 x cw natural weight tile):
  absmax reduce on GPSIMD; normalize to an int16 grid (vn = w *
  32760/absmax) on GPSIMD/DVE; the 15-boundary NF4 bucketize runs as a
  sum of step terms  acc = sum_j (vn > IB_j) * IDELT_j  with the
  compares split across three engines:
    - DVE:    fused tensor_scalar (is_gt, mult) at 4x int16 rate
    - GPSIMD: plain is_gt masks {0,1}
    - ACT:    Sign(vn - (IB_j+0.5)) giving exact {-1,+1} (deltas are
              even so the +-IDELT/2 folds stay integral)
  all folded into an int16 acc via fused scalar_tensor_tensor on DVE.
  Rescale (acc - offset) * absmax/16384 -> bf16, then XBAR DMA
  transposes (SBUF->SBUF) produce lhsT-layout tiles which are stored
  to DRAM scratch with one batched DMA (no PE transposes, no PSUM
  evict copies).

Phase2 processes token-blocks in pairs (2 x 512) so each s1/s2 strip
load feeds two blocks and each lhsT stationary load feeds 2x512
columns.  Phase3 loads each s3 slice once per (tb, dc) and shares it
across both 4-bank PSUM half-groups.

Env knobs: KSPLIT="d,g,a" (# compares on DVE/GPSIMD/ACT),
KERNEL_GPS_NORM, KERNEL_GPS_RESCALE, KERNEL_GPS_REDUCE (0|1).
"""

import os
import sys

import numpy as np

if not os.path.isdir(os.path.join(os.path.dirname(os.path.abspath(__file__)), "concourse")):
    for _p in ("/opt/trn_rl_repo",):
        if os.path.isdir(_p) and _p not in sys.path:
            sys.path.insert(0, _p)

import ml_dtypes

import concourse.bass as bass
import concourse.mybir as mybir
import concourse.tile as tile
from concourse import bacc
from concourse.bass_utils import run_bass_kernel_spmd

F32 = mybir.dt.float32
F16 = mybir.dt.float16
BF16 = mybir.dt.bfloat16
I16 = mybir.dt.int16
OP = mybir.AluOpType
AF = mybir.ActivationFunctionType

NF4_CODE = np.array(
    [
        -1.0, -0.6961928009986877, -0.5250730514526367, -0.39491748809814453,
        -0.28444138169288635, -0.18477343022823334, -0.09105003625154495, 0.0,
        0.07958029955625534, 0.16093020141124725, 0.24611230194568634,
        0.33791524171829224, 0.44070982933044434, 0.5626170039176941,
        0.7229568362236023, 1.0,
    ],
    dtype=np.float64,
)
NF4_BOUNDS = (NF4_CODE[:-1] + NF4_CODE[1:]) * 0.5

CSCALE = 32760.0  # int16 compare-domain scale (saturation-safe)
IBOUND = [int(np.floor(b * CSCALE)) for b in NF4_BOUNDS]
# Even cumulative code table at scale 16384: CUM[j] ~ (code_j + 1) * 16384.
CUM = [2 * int(round((c + 1.0) * 8192.0)) for c in NF4_CODE]
IDELT = [CUM[j + 1] - CUM[j] for j in range(15)]  # all even
VSCALE = 16384.0

BLK = 64

D = 4096
T_FULL = 4096
H_FULL = 11008
N_CORES = 8
HP = 1408
SHARD_W = [1408, 1408, 1408, 1408, 1344, 1344, 1344, 1344]
SHARD_START = [0, 1408, 2816, 4224, 5632, 6976, 8320, 9664]

KT = D // 128  # 32
HT = HP // 128  # 11
TBP = 512           # phase2 token block (processed in pairs)
NPAIR = T_FULL // (2 * TBP)  # 4
T3 = 1024           # phase3 token block
SEG = 2048          # strip segment width (bf16)

DQ_CHUNK = 1024
W3_CHUNKS = [(0, 1024), (1024, 384)]

_split = os.environ.get("KSPLIT", "9,0,6")
N_DVE, N_GPS, N_ACT = [int(v) for v in _split.split(",")]
assert N_DVE + N_GPS + N_ACT == 15
DVE_J = list(range(0, N_DVE))
GPS_J = list(range(N_DVE, N_DVE + N_GPS))
ACT_J = list(range(N_DVE + N_GPS, 15))
S_HALF = sum(IDELT[j] // 2 for j in ACT_J)
OFFSET = 16384 - S_HALF  # dq = (acc - OFFSET) * absmax/16384

# Pool (gpsimd) integer ops require matching dtypes on all operands, so the
# f32->i16 normalize and i16xf32->bf16 rescale must stay on DVE.
GPS_NORM = os.environ.get("KERNEL_GPS_NORM", "0") == "1"
GPS_RESCALE = os.environ.get("KERNEL_GPS_RESCALE", "0") == "1"
# gpsimd tensor_reduce is partition-axis only; free-dim absmax must run on DVE
GPS_REDUCE = os.environ.get("KERNEL_GPS_REDUCE", "0") == "1"


class P:
    pass


def _build_program():
    nc = bacc.Bacc("TRN2", target_bir_lowering=False, debug=False, num_devices=N_CORES)

    xT = nc.dram_tensor("xT", [D, T_FULL], BF16, kind="ExternalInput").ap()
    w1s = nc.dram_tensor("w1s", [HP, D], F32, kind="ExternalInput").ap()
    w2s = nc.dram_tensor("w2s", [HP, D], F32, kind="ExternalInput").ap()
    w3s = nc.dram_tensor("w3s", [D, HP], F32, kind="ExternalInput").ap()
    out = nc.dram_tensor("out", [T_FULL, D], F32, kind="ExternalOutput").ap()

    from contextlib import ExitStack

    with tile.TileContext(nc) as tc, ExitStack() as ctx:
        p = P()
        dram = ctx.enter_context(tc.tile_pool(name="dram", bufs=1, space="DRAM"))
        s1 = dram.tile([HT, 128, KT, 128], BF16)
        s2 = dram.tile([HT, 128, KT, 128], BF16)
        s3 = dram.tile([HT, 128, D], BF16)
        hTd = dram.tile([HT, 128, T_FULL], BF16)

        const = ctx.enter_context(tc.tile_pool(name="const", bufs=1))
        bias_aps = []
        for j in range(15):
            b = const.tile([128, 1], F32, tag=f"bias{j}", name=f"bias{j}")
            nc.vector.memset(b[:], -(IBOUND[j] + 0.5))
            bias_aps.append(b)

        pool_spec = [
            ("pxb", 64, [128, TBP], BF16),
            ("pw", 2, [128, DQ_CHUNK], F32),
            ("pa", 8, [128, DQ_CHUNK // BLK], F32),
            ("pvn", 2, [128, DQ_CHUNK], I16),
            ("pmask", 14, [128, DQ_CHUNK], I16),
            ("pprod", 2, [128, DQ_CHUNK], I16),
            ("pchain", 2, [128, DQ_CHUNK], I16),
            ("pdq", 3, [128, DQ_CHUNK], BF16),
            ("pqt", 4, [128, DQ_CHUNK], BF16),
            ("pl", 3, [128, SEG], BF16),  # bufs are per-tag (l1, l2)
            ("pht", 3, [128, TBP], BF16),
            ("psl", 2, [128, TBP], BF16),
            ("pue", 2, [128, TBP], BF16),
            ("phl", 11, [128, T3], BF16),
            ("pr3", 12, [128, 512], BF16),
            ("pob", 3, [128, 512], F32),
        ]
        for nm, bufs, shape, dt in pool_spec:
            setattr(p, nm, ctx.enter_context(tc.tile_pool(name=nm, bufs=bufs)))
        p.pps = ctx.enter_context(tc.tile_pool(name="pps", bufs=8, space="PSUM"))

        # ---------------- dequant one [128, cw] natural tile ----------------
        def dq_tile(w_ap, row0, col0, cw, store_fn):
            nblk = cw // BLK
            wt = p.pw.tile([128, cw], F32, tag="wt", name="wt")
            nc.sync.dma_start(wt[:], w_ap[row0 : row0 + 128, col0 : col0 + cw])
            w3v = wt[:].rearrange("p (b i) -> p b i", i=BLK)

            amax = p.pa.tile([128, nblk], F32, tag="amax", name="amax")
            red_eng = nc.gpsimd if GPS_REDUCE else nc.vector
            red_eng.tensor_reduce(
                amax[:], w3v, axis=mybir.AxisListType.X, op=OP.max,
                apply_absolute_value=True,
            )
            aclamp = p.pa.tile([128, nblk], F32, tag="aclamp", name="aclamp")
            nc.vector.tensor_scalar_max(aclamp[:], amax[:], 1e-30)
            recip = p.pa.tile([128, nblk], F32, tag="recip", name="recip")
            nc.vector.reciprocal(recip[:], aclamp[:])
            rs = p.pa.tile([128, nblk], F32, tag="rs", name="rs")
            nc.vector.tensor_scalar_mul(rs[:], recip[:], CSCALE)
            # av must stay f32: amax/16384 ~ 3e-7 underflows fp16 normals
            av = p.pa.tile([128, nblk], F32, tag="av", name="av")
            nc.vector.tensor_scalar_mul(av[:], amax[:], 1.0 / VSCALE)

            r_b = rs[:].unsqueeze(2).broadcast_to([128, nblk, BLK])
            av_b = av[:].unsqueeze(2).broadcast_to([128, nblk, BLK])

            vn = p.pvn.tile([128, cw], I16, tag="vn", name="vn")
            vn3 = vn[:].rearrange("p (b i) -> p b i", i=BLK)
            norm_eng = nc.gpsimd if GPS_NORM else nc.vector
            norm_eng.tensor_tensor(vn3, w3v, r_b, OP.mult)

            # compares on GPS / ACT (consumed later by DVE folds).
            # Scalars are passed as FLOATS: integer immediates push DVE
            # tensor_scalar/STT onto a ~6-11x slower integer path.
            scaled_tiles = []
            for j in GPS_J:
                # fused (is_gt, mult) -> {0, IDELT} directly on gpsimd
                m = p.pmask.tile([128, cw], I16, tag="mask", name=f"g{j}")
                nc.gpsimd.tensor_scalar(
                    m[:], vn[:], float(IBOUND[j]), float(IDELT[j]), OP.is_gt, OP.mult
                )
                scaled_tiles.append(m)
            for j in ACT_J:
                # Sign -> {-1,+1}, then ACT-side scale -> {-IDELT/2, +IDELT/2};
                # the half-delta offsets fold into OFFSET.
                sg = p.pmask.tile([128, cw], I16, tag="mask", name=f"a{j}")
                nc.scalar.activation(sg[:], vn[:], AF.Sign, bias=bias_aps[j][:])
                sg2 = p.pmask.tile([128, cw], I16, tag="mask", name=f"as{j}")
                nc.scalar.activation(
                    sg2[:], sg[:], AF.Copy, bias=0.0, scale=float(IDELT[j] // 2)
                )
                scaled_tiles.append(sg2)

            # DVE chain: fused compares + plain 2x-rate int16 adds.
            # DVE-local terms accumulate independently of the ACT-produced
            # tiles so the DVE never stalls waiting for ACT mid-chain.
            acc = p.pchain.tile([128, cw], I16, tag="acc", name="acc")
            first = True
            for j in DVE_J:
                if first:
                    nc.vector.tensor_scalar(
                        acc[:], vn[:], float(IBOUND[j]), float(IDELT[j]),
                        OP.is_gt, OP.mult,
                    )
                    first = False
                else:
                    t = p.pprod.tile([128, cw], I16, tag="prod", name="t")
                    nc.vector.tensor_scalar(
                        t[:], vn[:], float(IBOUND[j]), float(IDELT[j]),
                        OP.is_gt, OP.mult,
                    )
                    nc.vector.tensor_tensor(acc[:], acc[:], t[:], OP.add)
            if scaled_tiles:
                if len(scaled_tiles) >= 2:
                    acc2 = p.pchain.tile([128, cw], I16, tag="acc2", name="acc2")
                    nc.vector.tensor_tensor(
                        acc2[:], scaled_tiles[0][:], scaled_tiles[1][:], OP.add
                    )
                    for m in scaled_tiles[2:]:
                        nc.vector.tensor_tensor(acc2[:], acc2[:], m[:], OP.add)
                    rest = acc2
                else:
                    rest = scaled_tiles[0]
                if first:
                    nc.vector.tensor_copy(acc[:], rest[:])
                    first = False
                else:
                    nc.vector.tensor_tensor(acc[:], acc[:], rest[:], OP.add)

            dq = p.pdq.tile([128, cw], BF16, tag="dq", name="dq")
            dq3 = dq[:].rearrange("p (b i) -> p b i", i=BLK)
            acc3 = acc[:].rearrange("p (b i) -> p b i", i=BLK)
            resc_eng = nc.gpsimd if GPS_RESCALE else nc.vector
            resc_eng.scalar_tensor_tensor(
                dq3, acc3, float(-OFFSET), av_b, OP.add, OP.mult
            )

            qt = p.pqt.tile([128, cw], BF16, tag="qt", name="qt")
            for jb in range(cw // 128):
                sl = slice(jb * 128, (jb + 1) * 128)
                nc.sync.dma_start_transpose(qt[:, sl], dq[:, sl])
            store_fn(qt)

        def dq_w12(which, s, h):
            w_ap = w1s if which == 1 else w2s
            for ch in range(0, D, DQ_CHUNK):
                def store(qt, ch=ch, h=h, s=s):
                    kt0 = ch // 128
                    nkt = DQ_CHUNK // 128
                    dst = s[h, :, kt0 : kt0 + nkt, :].rearrange("p k i -> p (k i)")
                    nc.gpsimd.dma_start(dst, qt[:])
                dq_tile(w_ap, h * 128, ch, DQ_CHUNK, store)

        w3_work = [(i, ch, cw) for i in range(KT) for (ch, cw) in W3_CHUNKS]
        w3_iter = iter(w3_work)

        def emit_w3(n):
            for _ in range(n):
                item = next(w3_iter, None)
                if item is None:
                    return
                i, ch, cw = item
                def store(qt, i=i, ch=ch, cw=cw):
                    for jb in range(cw // 128):
                        hb = ch // 128 + jb
                        nc.gpsimd.dma_start(
                            s3[hb, :, i * 128 : (i + 1) * 128],
                            qt[:, jb * 128 : (jb + 1) * 128],
                        )
                dq_tile(w3s, i * 128, ch, cw, store)

        # ---------------- phase 2 ----------------
        def load_x(tb):
            xk = []
            for k in range(KT):
                xf = p.pxb.tile([128, TBP], BF16, tag="xb", name="xb")
                nc.sync.dma_start(
                    xf[:], xT[k * 128 : (k + 1) * 128, tb * TBP : (tb + 1) * TBP]
                )
                xk.append(xf)
            return xk

        def load_strip(s, h, tag):
            segs = []
            for k0 in range(0, KT * 128, SEG):
                seg = p.pl.tile([128, SEG], BF16, tag=tag, name=tag)
                nc.sync.dma_start(
                    seg[:],
                    s[h, :, k0 // 128 : (k0 + SEG) // 128, :].rearrange(
                        "p k i -> p (k i)"
                    ),
                )
                segs.append(seg)
            return segs

        def lhs_slice(segs, k):
            o = (k * 128) % SEG
            return segs[(k * 128) // SEG][:, o : o + 128]

        def phase2_pair(pair, h, xka, xkb):
            l1 = load_strip(s1, h, "l1")
            l2 = load_strip(s2, h, "l2")
            pg_a = p.pps.tile([128, TBP], F32, tag="ps", name="pg_a")
            pg_b = p.pps.tile([128, TBP], F32, tag="ps", name="pg_b")
            for k in range(KT):
                sl_ap = lhs_slice(l1, k)
                nc.tensor.matmul(pg_a[:], sl_ap, xka[k][:], start=(k == 0), stop=(k == KT - 1))
                nc.tensor.matmul(pg_b[:], sl_ap, xkb[k][:], start=(k == 0), stop=(k == KT - 1))
            pu_a = p.pps.tile([128, TBP], F32, tag="ps", name="pu_a")
            pu_b = p.pps.tile([128, TBP], F32, tag="ps", name="pu_b")
            for k in range(KT):
                sl_ap = lhs_slice(l2, k)
                nc.tensor.matmul(pu_a[:], sl_ap, xka[k][:], start=(k == 0), stop=(k == KT - 1))
                nc.tensor.matmul(pu_b[:], sl_ap, xkb[k][:], start=(k == 0), stop=(k == KT - 1))
            for pg, pu, tb in ((pg_a, pu_a, 2 * pair), (pg_b, pu_b, 2 * pair + 1)):
                slt = p.psl.tile([128, TBP], BF16, tag="sl", name="sl")
                nc.scalar.activation(slt[:], pg[:], AF.Silu)
                ue = p.pue.tile([128, TBP], BF16, tag="ue", name="ue")
                nc.scalar.copy(ue[:], pu[:])
                ht = p.pht.tile([128, TBP], BF16, tag="ht", name="ht")
                nc.gpsimd.tensor_tensor(ht[:], slt[:], ue[:], OP.mult)
                nc.gpsimd.dma_start(hTd[h, :, tb * TBP : (tb + 1) * TBP], ht[:])

        # ---------------- phase 3 ----------------
        def phase3(tb3):
            strips = []
            for k in range(HT):
                hl = p.phl.tile([128, T3], BF16, tag="hl", name="hl")
                nc.sync.dma_start(hl[:], hTd[k, :, tb3 * T3 : (tb3 + 1) * T3])
                strips.append(hl)
            for dc in range(D // 512):
                r3s = []
                for k in range(HT):
                    r3 = p.pr3.tile([128, 512], BF16, tag="r3", name="r3")
                    nc.sync.dma_start(r3[:], s3[k, :, dc * 512 : (dc + 1) * 512])
                    r3s.append(r3)
                for th in range(2):
                    po = [
                        p.pps.tile([128, 512], F32, tag="ps", name=f"po{tt}")
                        for tt in range(4)
                    ]
                    for k in range(HT):
                        for i in range(4):
                            tt = th * 4 + i
                            nc.tensor.matmul(
                                po[i][:],
                                strips[k][:, tt * 128 : (tt + 1) * 128],
                                r3s[k][:],
                                start=(k == 0), stop=(k == HT - 1),
                            )
                    for i in range(4):
                        tt = th * 4 + i
                        ob = p.pob.tile([128, 512], F32, tag="ob", name="ob")
                        nc.scalar.copy(ob[:], po[i][:])
                        nc.gpsimd.dma_start(
                            out[
                                tb3 * T3 + tt * 128 : tb3 * T3 + (tt + 1) * 128,
                                dc * 512 : (dc + 1) * 512,
                            ],
                            ob[:],
                        )

        # ---------------- main flow ----------------
        # w3 dequant is front-loaded into pairs 1-2 so s3 completes before
        # pair 3; phase3(0,1) then interleaves ahead of pair 3's matmuls.
        # Lag-1 emission: each h's phase2 is emitted BEFORE the next h's
        # dequant batch, so the silu/ue PSUM evictions sit ahead of the
        # dequant ops in the ACT queue and PSUM banks recycle promptly.
        # All w3 dequant is packed into pair 1 so s3 completes early, and
        # phase3 blocks are interleaved between pairs 2/3 to keep the PE
        # busy while the dequant chain drains.
        w3_per_pair = {1: 6, 2: 0, 3: 0}
        prev = None  # (pair, h, xka, xkb)
        for pair in range(NPAIR):
            xka = load_x(2 * pair)
            xkb = load_x(2 * pair + 1)
            for h in range(HT):
                if pair == 0:
                    dq_w12(1, s1, h)
                    dq_w12(2, s2, h)
                else:
                    emit_w3(w3_per_pair[pair])
                if prev is not None:
                    phase2_pair(*prev)
                prev = (pair, h, xka, xkb)
            if pair == 1:
                emit_w3(len(w3_work))  # drain any remainder
                phase2_pair(*prev)
                prev = None
                phase3(0)
            elif pair == 2:
                phase2_pair(*prev)
                prev = None
                phase3(1)
                phase3(2)
        phase2_pair(*prev)
        phase3(3)

    nc.compile()
    return nc


_CACHED_NC = None
LAST_RESULTS = None


def _shard_inputs(x, w1, w2, w3):
    xT16 = np.ascontiguousarray(
        x.reshape(T_FULL, D).T.astype(ml_dtypes.bfloat16)
    )
    in_maps = []
    for c in range(N_CORES):
        s, w = SHARD_START[c], SHARD_W[c]
        w1c = np.zeros((HP, D), dtype=np.float32)
        w1c[:w] = w1[s : s + w]
        w2c = np.zeros((HP, D), dtype=np.float32)
        w2c[:w] = w2[s : s + w]
        w3c = np.zeros((D, HP), dtype=np.float32)
        w3c[:, :w] = w3[:, s : s + w]
        in_maps.append({"xT": xT16, "w1s": w1c, "w2s": w2c, "w3s": w3c})
    return in_maps


def kernel(x, w1, w2, w3):
    global _CACHED_NC, LAST_RESULTS
    assert x.shape == (2, 2048, D) and w1.shape == (H_FULL, D)
    if _CACHED_NC is None:
        _CACHED_NC = _build_program()
    in_maps = _shard_inputs(x, w1, w2, w3)
    res = run_bass_kernel_spmd(
        _CACHED_NC,
        in_maps,
        core_ids=list(range(N_CORES)),
        trace=os.environ.get("KERNEL_TRACE", "") == "1",
    )
    LAST_RESULTS = res
    acc = res.results[0]["out"].astype(np.float32).copy()
    for c in range(1, N_CORES):
        acc += res.results[c]["out"]
    return acc.reshape(2, 2048, D).astype(np.float32)


# revision 27
# speedup vs baseline: 1.0813x; 1.0813x over previous
"""Bass/Trainium2 kernel for NF4-dequant (QLoRA-style) SwiGLU MLP.

Computation (matches the bitsandbytes-NF4 reference):
    dq_i = nf4_quant_dequant(w_i)   (per-64-block absmax scaling)
    out  = dq3-proj( silu(x @ dq1^T) * (x @ dq2^T) )

Sharding: tensor-parallel over the ffn dim H=11008 across 8 cores.
H is split in 64-aligned shards of width [1408 x4, 1344 x4]; the 1344
shards are zero-padded to 1408 so every core runs the same program.
Each core computes a full [T, D] partial of the down-projection; the
host sums the 8 partials (the TP all-reduce).

Dequant strategy (per 128 x cw natural weight tile):
  absmax reduce on GPSIMD; normalize to an int16 grid (vn = w *
  32760/absmax) on GPSIMD/DVE; the 15-boundary NF4 bucketize runs as a
  sum of step terms  acc = sum_j (vn > IB_j) * IDELT_j  with the
  compares split across three engines:
    - DVE:    fused tensor_scalar (is_gt, mult) at 4x int16 rate
    - GPSIMD: plain is_gt masks {0,1}
    - ACT:    Sign(vn - (IB_j+0.5)) giving exact {-1,+1} (deltas are
              even so the +-IDELT/2 folds stay integral)
  all folded into an int16 acc via fused scalar_tensor_tensor on DVE.
  Rescale (acc - offset) * absmax/16384 -> bf16, then XBAR DMA
  transposes (SBUF->SBUF) produce lhsT-layout tiles which are stored
  to DRAM scratch with one batched DMA (no PE transposes, no PSUM
  evict copies).

Phase2 processes token-blocks in pairs (2 x 512) so each s1/s2 strip
load feeds two blocks and each lhsT stationary load feeds 2x512
columns.  Phase3 loads each s3 slice once per (tb, dc) and shares it
across both 4-bank PSUM half-groups.

Env knobs: KSPLIT="d,g,a" (# compares on DVE/GPSIMD/ACT),
KERNEL_GPS_NORM, KERNEL_GPS_RESCALE, KERNEL_GPS_REDUCE (0|1).
"""

import os
import sys

import numpy as np

if not os.path.isdir(os.path.join(os.path.dirname(os.path.abspath(__file__)), "concourse")):
    for _p in ("/opt/trn_rl_repo",):
        if os.path.isdir(_p) and _p not in sys.path:
            sys.path.insert(0, _p)

import ml_dtypes

import concourse.bass as bass
import concourse.mybir as mybir
import concourse.tile as tile
from concourse import bacc
from concourse.bass_utils import run_bass_kernel_spmd

F32 = mybir.dt.float32
F16 = mybir.dt.float16
BF16 = mybir.dt.bfloat16
I16 = mybir.dt.int16
OP = mybir.AluOpType
AF = mybir.ActivationFunctionType

NF4_CODE = np.array(
    [
        -1.0, -0.6961928009986877, -0.5250730514526367, -0.39491748809814453,
        -0.28444138169288635, -0.18477343022823334, -0.09105003625154495, 0.0,
        0.07958029955625534, 0.16093020141124725, 0.24611230194568634,
        0.33791524171829224, 0.44070982933044434, 0.5626170039176941,
        0.7229568362236023, 1.0,
    ],
    dtype=np.float64,
)
NF4_BOUNDS = (NF4_CODE[:-1] + NF4_CODE[1:]) * 0.5

CSCALE = 32760.0  # int16 compare-domain scale (saturation-safe)
IBOUND = [int(np.floor(b * CSCALE)) for b in NF4_BOUNDS]
# Even cumulative code table at scale 16384: CUM[j] ~ (code_j + 1) * 16384.
CUM = [2 * int(round((c + 1.0) * 8192.0)) for c in NF4_CODE]
IDELT = [CUM[j + 1] - CUM[j] for j in range(15)]  # all even
VSCALE = 16384.0

BLK = 64

D = 4096
T_FULL = 4096
H_FULL = 11008
N_CORES = 8
HP = 1408
SHARD_W = [1408, 1408, 1408, 1408, 1344, 1344, 1344, 1344]
SHARD_START = [0, 1408, 2816, 4224, 5632, 6976, 8320, 9664]

KT = D // 128  # 32
HT = HP // 128  # 11
TBP = 512           # phase2 token block (processed in pairs)
NPAIR = T_FULL // (2 * TBP)  # 4
T3 = 1024           # phase3 token block
SEG = 2048          # strip segment width (bf16)

DQ_CHUNK = 1024
W3_CHUNKS = [(0, 1024), (1024, 384)]

_split = os.environ.get("KSPLIT", "8,0,7")
N_DVE, N_GPS, N_ACT = [int(v) for v in _split.split(",")]
assert N_DVE + N_GPS + N_ACT == 15
DVE_J = list(range(0, N_DVE))
GPS_J = list(range(N_DVE, N_DVE + N_GPS))
ACT_J = list(range(N_DVE + N_GPS, 15))
S_HALF = sum(IDELT[j] // 2 for j in ACT_J)
OFFSET = 16384 - S_HALF  # dq = (acc - OFFSET) * absmax/16384

# Pool (gpsimd) integer ops require matching dtypes on all operands, so the
# f32->i16 normalize and i16xf32->bf16 rescale must stay on DVE.
GPS_NORM = os.environ.get("KERNEL_GPS_NORM", "0") == "1"
GPS_RESCALE = os.environ.get("KERNEL_GPS_RESCALE", "0") == "1"
# gpsimd tensor_reduce is partition-axis only; free-dim absmax must run on DVE
GPS_REDUCE = os.environ.get("KERNEL_GPS_REDUCE", "0") == "1"


class P:
    pass


def _build_program():
    nc = bacc.Bacc("TRN2", target_bir_lowering=False, debug=False, num_devices=N_CORES)

    xT = nc.dram_tensor("xT", [D, T_FULL], BF16, kind="ExternalInput").ap()
    w1s = nc.dram_tensor("w1s", [HP, D], F32, kind="ExternalInput").ap()
    w2s = nc.dram_tensor("w2s", [HP, D], F32, kind="ExternalInput").ap()
    w3s = nc.dram_tensor("w3s", [D, HP], F32, kind="ExternalInput").ap()
    out = nc.dram_tensor("out", [T_FULL, D], F32, kind="ExternalOutput").ap()

    from contextlib import ExitStack

    with tile.TileContext(nc) as tc, ExitStack() as ctx:
        p = P()
        dram = ctx.enter_context(tc.tile_pool(name="dram", bufs=1, space="DRAM"))
        s1 = dram.tile([HT, 128, KT, 128], BF16)
        s2 = dram.tile([HT, 128, KT, 128], BF16)
        s3 = dram.tile([HT, 128, D], BF16)
        hTd = dram.tile([HT, 128, T_FULL], BF16)

        const = ctx.enter_context(tc.tile_pool(name="const", bufs=1))
        bias_aps = []
        for j in range(15):
            b = const.tile([128, 1], F32, tag=f"bias{j}", name=f"bias{j}")
            nc.vector.memset(b[:], -(IBOUND[j] + 0.5))
            bias_aps.append(b)
        from concourse.masks import make_identity
        ident = const.tile([128, 128], F16, tag="ident", name="ident")
        make_identity(nc, ident[:])
        diag_aps = {}
        for j in ACT_J:
            dgt = const.tile([128, 128], F16, tag=f"diag{j}", name=f"diag{j}")
            nc.vector.tensor_scalar_mul(dgt[:], ident[:], float(IDELT[j] // 2))
            diag_aps[j] = dgt

        pool_spec = [
            ("pxb", 64, [128, TBP], BF16),
            ("pw", 2, [128, DQ_CHUNK], F32),
            ("pa", 8, [128, DQ_CHUNK // BLK], F32),
            ("pvn", 2, [128, DQ_CHUNK], I16),
            ("pmask", 14, [128, DQ_CHUNK], I16),
            ("pprod", 2, [128, DQ_CHUNK], I16),
            ("pchain", 2, [128, DQ_CHUNK], I16),
            ("pdq", 3, [128, DQ_CHUNK], BF16),
            ("pqt", 4, [128, DQ_CHUNK], BF16),
            ("pl", 3, [128, SEG], BF16),  # bufs are per-tag (l1, l2)
            ("pht", 3, [128, TBP], BF16),
            ("psl", 2, [128, TBP], BF16),
            ("pue", 2, [128, TBP], BF16),
            ("phl", 11, [128, T3], BF16),
            ("pr3", 12, [128, 512], BF16),
            ("pob", 3, [128, 512], F32),
        ]
        for nm, bufs, shape, dt in pool_spec:
            setattr(p, nm, ctx.enter_context(tc.tile_pool(name=nm, bufs=bufs)))
        p.pps = ctx.enter_context(tc.tile_pool(name="pps", bufs=8, space="PSUM"))

        # ---------------- dequant one [128, cw] natural tile ----------------
        def dq_tile(w_ap, row0, col0, cw, store_fn):
            nblk = cw // BLK
            wt = p.pw.tile([128, cw], F32, tag="wt", name="wt")
            nc.sync.dma_start(wt[:], w_ap[row0 : row0 + 128, col0 : col0 + cw])
            w3v = wt[:].rearrange("p (b i) -> p b i", i=BLK)

            amax = p.pa.tile([128, nblk], F32, tag="amax", name="amax")
            red_eng = nc.gpsimd if GPS_REDUCE else nc.vector
            red_eng.tensor_reduce(
                amax[:], w3v, axis=mybir.AxisListType.X, op=OP.max,
                apply_absolute_value=True,
            )
            aclamp = p.pa.tile([128, nblk], F32, tag="aclamp", name="aclamp")
            nc.vector.tensor_scalar_max(aclamp[:], amax[:], 1e-30)
            recip = p.pa.tile([128, nblk], F32, tag="recip", name="recip")
            nc.vector.reciprocal(recip[:], aclamp[:])
            rs = p.pa.tile([128, nblk], F32, tag="rs", name="rs")
            nc.vector.tensor_scalar_mul(rs[:], recip[:], CSCALE)
            # av must stay f32: amax/16384 ~ 3e-7 underflows fp16 normals
            av = p.pa.tile([128, nblk], F32, tag="av", name="av")
            nc.vector.tensor_scalar_mul(av[:], amax[:], 1.0 / VSCALE)

            r_b = rs[:].unsqueeze(2).broadcast_to([128, nblk, BLK])
            av_b = av[:].unsqueeze(2).broadcast_to([128, nblk, BLK])

            vn = p.pvn.tile([128, cw], I16, tag="vn", name="vn")
            vn3 = vn[:].rearrange("p (b i) -> p b i", i=BLK)
            norm_eng = nc.gpsimd if GPS_NORM else nc.vector
            norm_eng.tensor_tensor(vn3, w3v, r_b, OP.mult)

            # compares on GPS / ACT (consumed later by DVE folds).
            # Scalars are passed as FLOATS: integer immediates push DVE
            # tensor_scalar/STT onto a ~6-11x slower integer path.
            # ACT terms: Sign -> exact {-1,+1} bf16 tiles; they are folded on
            # the (mostly idle) PE as sum_j diag(IDELT_j/2) @ sign_j
            # accumulated in PSUM.  The half-delta offsets fold into OFFSET.
            sign_tiles = []
            for j in ACT_J:
                sg = p.pmask.tile([128, cw], BF16, tag="mask", name=f"a{j}")
                nc.scalar.activation(sg[:], vn[:], AF.Sign, bias=bias_aps[j][:])
                sign_tiles.append((j, sg))
            fold_halves = []
            for h0 in range(0, cw, 512):
                hw = min(512, cw - h0)
                ps = p.pps.tile([128, hw], F32, tag="ps", name="fold")
                for i, (j, sg) in enumerate(sign_tiles):
                    nc.tensor.matmul(
                        ps[:], diag_aps[j][:], sg[:, h0 : h0 + hw],
                        start=(i == 0), stop=(i == len(sign_tiles) - 1),
                    )
                fold_halves.append((h0, hw, ps))

            # DVE chain: fused compares + plain 2x-rate int16 adds.
            # DVE-local terms accumulate independently of the ACT-produced
            # tiles so the DVE never stalls waiting for ACT mid-chain.
            acc = p.pchain.tile([128, cw], I16, tag="acc", name="acc")
            first = True
            for j in DVE_J:
                if first:
                    nc.vector.tensor_scalar(
                        acc[:], vn[:], float(IBOUND[j]), float(IDELT[j]),
                        OP.is_gt, OP.mult,
                    )
                    first = False
                else:
                    t = p.pprod.tile([128, cw], I16, tag="prod", name="t")
                    nc.vector.tensor_scalar(
                        t[:], vn[:], float(IBOUND[j]), float(IDELT[j]),
                        OP.is_gt, OP.mult,
                    )
                    nc.vector.tensor_tensor(acc[:], acc[:], t[:], OP.add)

            dq = p.pdq.tile([128, cw], BF16, tag="dq", name="dq")
            for h0, hw, ps in fold_halves:
                # ps += acc_i16 - OFFSET  (in-place PSUM merge on DVE)
                nc.vector.scalar_tensor_tensor(
                    ps[:], acc[:, h0 : h0 + hw], float(-OFFSET), ps[:],
                    OP.add, OP.add,
                )
                nb0, nbw = h0 // BLK, hw // BLK
                dqv = dq[:, h0 : h0 + hw].rearrange("p (b i) -> p b i", i=BLK)
                psv = ps[:].rearrange("p (b i) -> p b i", i=BLK)
                nc.vector.tensor_tensor(
                    dqv, psv, av_b[:, nb0 : nb0 + nbw, :], OP.mult
                )

            qt = p.pqt.tile([128, cw], BF16, tag="qt", name="qt")
            for jb in range(cw // 128):
                sl = slice(jb * 128, (jb + 1) * 128)
                nc.sync.dma_start_transpose(qt[:, sl], dq[:, sl])
            store_fn(qt)

        def dq_w12(which, s, h):
            w_ap = w1s if which == 1 else w2s
            for ch in range(0, D, DQ_CHUNK):
                def store(qt, ch=ch, h=h, s=s):
                    kt0 = ch // 128
                    nkt = DQ_CHUNK // 128
                    dst = s[h, :, kt0 : kt0 + nkt, :].rearrange("p k i -> p (k i)")
                    nc.gpsimd.dma_start(dst, qt[:])
                dq_tile(w_ap, h * 128, ch, DQ_CHUNK, store)

        w3_work = [(i, ch, cw) for i in range(KT) for (ch, cw) in W3_CHUNKS]
        w3_iter = iter(w3_work)

        def emit_w3(n):
            for _ in range(n):
                item = next(w3_iter, None)
                if item is None:
                    return
                i, ch, cw = item
                def store(qt, i=i, ch=ch, cw=cw):
                    for jb in range(cw // 128):
                        hb = ch // 128 + jb
                        nc.gpsimd.dma_start(
                            s3[hb, :, i * 128 : (i + 1) * 128],
                            qt[:, jb * 128 : (jb + 1) * 128],
                        )
                dq_tile(w3s, i * 128, ch, cw, store)

        # ---------------- phase 2 ----------------
        def load_x(tb):
            xk = []
            for k in range(KT):
                xf = p.pxb.tile([128, TBP], BF16, tag="xb", name="xb")
                nc.sync.dma_start(
                    xf[:], xT[k * 128 : (k + 1) * 128, tb * TBP : (tb + 1) * TBP]
                )
                xk.append(xf)
            return xk

        def load_strip(s, h, tag):
            segs = []
            for k0 in range(0, KT * 128, SEG):
                seg = p.pl.tile([128, SEG], BF16, tag=tag, name=tag)
                nc.sync.dma_start(
                    seg[:],
                    s[h, :, k0 // 128 : (k0 + SEG) // 128, :].rearrange(
                        "p k i -> p (k i)"
                    ),
                )
                segs.append(seg)
            return segs

        def lhs_slice(segs, k):
            o = (k * 128) % SEG
            return segs[(k * 128) // SEG][:, o : o + 128]

        def phase2_pair(pair, h, xka, xkb):
            l1 = load_strip(s1, h, "l1")
            l2 = load_strip(s2, h, "l2")
            pg_a = p.pps.tile([128, TBP], F32, tag="ps", name="pg_a")
            pg_b = p.pps.tile([128, TBP], F32, tag="ps", name="pg_b")
            for k in range(KT):
                sl_ap = lhs_slice(l1, k)
                nc.tensor.matmul(pg_a[:], sl_ap, xka[k][:], start=(k == 0), stop=(k == KT - 1))
                nc.tensor.matmul(pg_b[:], sl_ap, xkb[k][:], start=(k == 0), stop=(k == KT - 1))
            pu_a = p.pps.tile([128, TBP], F32, tag="ps", name="pu_a")
            pu_b = p.pps.tile([128, TBP], F32, tag="ps", name="pu_b")
            for k in range(KT):
                sl_ap = lhs_slice(l2, k)
                nc.tensor.matmul(pu_a[:], sl_ap, xka[k][:], start=(k == 0), stop=(k == KT - 1))
                nc.tensor.matmul(pu_b[:], sl_ap, xkb[k][:], start=(k == 0), stop=(k == KT - 1))
            for pg, pu, tb in ((pg_a, pu_a, 2 * pair), (pg_b, pu_b, 2 * pair + 1)):
                slt = p.psl.tile([128, TBP], BF16, tag="sl", name="sl")
                nc.scalar.activation(slt[:], pg[:], AF.Silu)
                ue = p.pue.tile([128, TBP], BF16, tag="ue", name="ue")
                nc.scalar.copy(ue[:], pu[:])
                ht = p.pht.tile([128, TBP], BF16, tag="ht", name="ht")
                nc.gpsimd.tensor_tensor(ht[:], slt[:], ue[:], OP.mult)
                nc.gpsimd.dma_start(hTd[h, :, tb * TBP : (tb + 1) * TBP], ht[:])

        # ---------------- phase 3 ----------------
        def phase3(tb3):
            strips = []
            for k in range(HT):
                hl = p.phl.tile([128, T3], BF16, tag="hl", name="hl")
                nc.sync.dma_start(hl[:], hTd[k, :, tb3 * T3 : (tb3 + 1) * T3])
                strips.append(hl)
            for dc in range(D // 512):
                r3s = []
                for k in range(HT):
                    r3 = p.pr3.tile([128, 512], BF16, tag="r3", name="r3")
                    nc.sync.dma_start(r3[:], s3[k, :, dc * 512 : (dc + 1) * 512])
                    r3s.append(r3)
                for th in range(2):
                    po = [
                        p.pps.tile([128, 512], F32, tag="ps", name=f"po{tt}")
                        for tt in range(4)
                    ]
                    for k in range(HT):
                        for i in range(4):
                            tt = th * 4 + i
                            nc.tensor.matmul(
                                po[i][:],
                                strips[k][:, tt * 128 : (tt + 1) * 128],
                                r3s[k][:],
                                start=(k == 0), stop=(k == HT - 1),
                            )
                    for i in range(4):
                        tt = th * 4 + i
                        ob = p.pob.tile([128, 512], F32, tag="ob", name="ob")
                        nc.scalar.copy(ob[:], po[i][:])
                        nc.gpsimd.dma_start(
                            out[
                                tb3 * T3 + tt * 128 : tb3 * T3 + (tt + 1) * 128,
                                dc * 512 : (dc + 1) * 512,
                            ],
                            ob[:],
                        )

        # ---------------- main flow ----------------
        # w3 dequant is front-loaded into pairs 1-2 so s3 completes before
        # pair 3; phase3(0,1) then interleaves ahead of pair 3's matmuls.
        # Lag-1 emission: each h's phase2 is emitted BEFORE the next h's
        # dequant batch, so the silu/ue PSUM evictions sit ahead of the
        # dequant ops in the ACT queue and PSUM banks recycle promptly.
        # All w3 dequant is packed into pair 1 so s3 completes early, and
        # phase3 blocks are interleaved between pairs 2/3 to keep the PE
        # busy while the dequant chain drains.
        w3_per_pair = {1: 4, 2: 2, 3: 0}
        prev = None  # (pair, h, xka, xkb)
        for pair in range(NPAIR):
            xka = load_x(2 * pair)
            xkb = load_x(2 * pair + 1)
            for h in range(HT):
                if pair == 0:
                    dq_w12(1, s1, h)
                    dq_w12(2, s2, h)
                else:
                    emit_w3(w3_per_pair[pair])
                if prev is not None:
                    phase2_pair(*prev)
                prev = (pair, h, xka, xkb)
            if pair == 2:
                emit_w3(len(w3_work))  # drain any remainder
                phase2_pair(*prev)
                prev = None
                phase3(0)
                phase3(1)
        phase2_pair(*prev)
        for tb3 in range(2, T_FULL // T3):
            phase3(tb3)

    nc.compile()
    return nc


_CACHED_NC = None
LAST_RESULTS = None


def _shard_inputs(x, w1, w2, w3):
    xT16 = np.ascontiguousarray(
        x.reshape(T_FULL, D).T.astype(ml_dtypes.bfloat16)
    )
    in_maps = []
    for c in range(N_CORES):
        s, w = SHARD_START[c], SHARD_W[c]
        w1c = np.zeros((HP, D), dtype=np.float32)
        w1c[:w] = w1[s : s + w]
        w2c = np.zeros((HP, D), dtype=np.float32)
        w2c[:w] = w2[s : s + w]
        w3c = np.zeros((D, HP), dtype=np.float32)
        w3c[:, :w] = w3[:, s : s + w]
        in_maps.append({"xT": xT16, "w1s": w1c, "w2s": w2c, "w3s": w3c})
    return in_maps


def kernel(x, w1, w2, w3):
    global _CACHED_NC, LAST_RESULTS
    assert x.shape == (2, 2048, D) and w1.shape == (H_FULL, D)
    if _CACHED_NC is None:
        _CACHED_NC = _build_program()
    in_maps = _shard_inputs(x, w1, w2, w3)
    res = run_bass_kernel_spmd(
        _CACHED_NC,
        in_maps,
        core_ids=list(range(N_CORES)),
        trace=os.environ.get("KERNEL_TRACE", "") == "1",
    )
    LAST_RESULTS = res
    acc = res.results[0]["out"].astype(np.float32).copy()
    for c in range(1, N_CORES):
        acc += res.results[c]["out"]
    return acc.reshape(2, 2048, D).astype(np.float32)
